# revision 7
# baseline (speedup 1.0000x reference)
"""Trainium2 Bass kernel for nn_CustomSimpleGRU (2-layer GRU-like recurrence).

Reference math (per timestep t):
    L0: gates = [x_t, h0] @ W0 + b0 ; z = sigmoid(gates[:, :H]) ; n = tanh(gates[:, 2H:3H])
        h0' = (1-z)*n + z*h0
    L1: gates = [h0', h1] @ W1 + b1 ; same gate math -> h1'
    out = h1'(last step) @ Wfc + bfc     (reset-gate chunk [H:2H] never used)

Sharding: data-parallel over batch (128 -> 16 per core x 8 cores), weights
replicated, time recurrence fully unrolled per core.

Per-core scheme ("column-packed" matmuls to beat the M=16 PE ceiling):
  - All matmuls use M=32 (batch 16 + 16 zero-padded lanes).
  - Gate columns are packed host-side as j-chunks: rhs_j = [z_j(256) | n_j(256)],
    j = 0..3, so one 512-wide matmul produces matching z/n slices.
  - For each j, the K-chunks (9 for L0, 16 for L1) are distributed over the
    four 32-row PE column groups via tile_position=(0, 32g): 4 concurrent
    accumulations -> psumA row-group g holds the partial sum of its K-subset.
  - The 4 partials are folded by one matmul with a 0/1 selection matrix
    Sel (Sel[p, b] = [p % 32 == b]) into row-group j of psumB: psumB then
    holds the complete layer gates for all four j-chunks, grouped
    [32*j + b] rows x [z(256) | n(256)] cols.
  - Elementwise (sigmoid/tanh/h-update) runs on full (128, 256) tiles
    (junk lanes compute zeros, discarded).
  - h state is kept in the same grouped layout h_g (128, 256):
    row 32*g + b, col f  =  h[b, 256*g + f].
  - Next step's stationary operands hT (128, 16) per 128-col h-chunk are made
    by 8 DMA-transposes per layer from h_g slices (off the PE).
  - Weight K-chunk c pairs hT chunk c = h cols [128c : 128c+128]; host packs
    W accordingly (natural order).
"""

import numpy as np

import concourse.bass as bass
import concourse.mybir as mybir
import concourse.tile as tile
from concourse import bacc

F32 = mybir.dt.float32
BF16 = mybir.dt.bfloat16
AF = mybir.ActivationFunctionType

B, S_FULL, IN, HID = 128, 512, 128, 1024
NCORES = 8
BL = B // NCORES  # 16 batch rows per core
NH = HID // 128  # 8 h-dim chunks
NJ = 4  # gate j-chunks, each [z 256 | n 256]
JW = 512  # packed j-chunk width
K0, K1 = 1 + NH, 2 * NH  # K-chunks per layer


def build_nc(S=S_FULL, with_bias=False):
    nc = bacc.Bacc("TRN2")
    xT_d = nc.dram_tensor("xT", [S, IN, BL], F32, kind="ExternalInput")
    w0_d = nc.dram_tensor("W0p", [K0, NJ, 128, JW], F32, kind="ExternalInput")
    w1_d = nc.dram_tensor("W1p", [K1, NJ, 128, JW], F32, kind="ExternalInput")
    id_d = nc.dram_tensor("Id", [128, 128], F32, kind="ExternalInput")
    wfc_d = nc.dram_tensor("Wfcp", [NH, 128], F32, kind="ExternalInput")
    bfc_d = nc.dram_tensor("bfc", [1], F32, kind="ExternalInput")
    b0_d = nc.dram_tensor("b0g", [128, JW], F32, kind="ExternalInput")
    b1_d = nc.dram_tensor("b1g", [128, JW], F32, kind="ExternalInput")
    o_d = nc.dram_tensor("o", [1, BL], F32, kind="ExternalOutput")

    with tile.TileContext(nc) as tc:
        with (
            tc.tile_pool(name="wts", bufs=1) as wts,
            tc.tile_pool(name="pb", bufs=2, space="PSUM") as pb_pool,
            tc.tile_pool(name="tp", bufs=2, space="PSUM") as tp_pool,
        ):
            stage_cm = tc.tile_pool(name="stage", bufs=2)
            stage = stage_cm.__enter__()
            # ---- weights fp32 DRAM -> bf16 SBUF ----
            w0_sb = wts.tile([128, K0, NJ, JW], BF16, tag="w0")
            w1_sb = wts.tile([128, K1, NJ, JW], BF16, tag="w1")
            for w_sb, w_d, kk in ((w0_sb, w0_d, K0), (w1_sb, w1_d, K1)):
                for k in range(kk):
                    for j in range(NJ):
                        st = stage.tile([128, JW], F32, tag="wstage")
                        nc.sync.dma_start(st[:], w_d[k, j])
                        nc.vector.tensor_copy(w_sb[:, k, j, :], st[:])
            id_sb = wts.tile([128, 128], BF16, tag="idm")
            id_st = stage.tile([128, 128], F32, tag="idst")
            nc.sync.dma_start(id_st[:], id_d[:])
            nc.vector.tensor_copy(id_sb[:], id_st[:])
            wfc_sb = wts.tile([128, NH], BF16, tag="wfc")
            wfc_st = stage.tile([128, NH], F32, tag="wfcst")
            wfc_ap = wfc_d[:]
            nc.sync.dma_start(
                wfc_st[:],
                bass.AP(tensor=wfc_ap.tensor, offset=0, ap=[[1, 128], [128, NH]]),
            )
            nc.vector.tensor_copy(wfc_sb[:], wfc_st[:])
            bfc_sb = wts.tile([1, 1], F32, tag="bfc")
            nc.sync.dma_start(bfc_sb[:], bfc_d[:])
            bias_sb = []
            if with_bias:
                for nm, b_d in (("b0", b0_d), ("b1", b1_d)):
                    bt = wts.tile([128, JW], F32, tag=f"bias{nm}", name=f"bias{nm}")
                    nc.sync.dma_start(bt[:], b_d[:])
                    bias_sb.append(bt)

            stage_cm.__exit__(None, None, None)
            state_cm = tc.tile_pool(name="state", bufs=2)
            state = state_cm.__enter__()
            tmp_cm = tc.tile_pool(name="tmp", bufs=2)
            tmp = tmp_cm.__enter__()
            xin_cm = tc.tile_pool(name="xin", bufs=4)
            xin = xin_cm.__enter__()

            # ---- initial state ----
            h0g = state.tile([128, 256], BF16, tag="h0g")
            h1g = state.tile([128, 256], BF16, tag="h1g")
            h0T = state.tile([128, NH, 32], BF16, tag="h0T")
            h1T = state.tile([128, NH, 32], BF16, tag="h1T")
            nc.vector.memset(h0g[:], 0.0)
            nc.vector.memset(h1g[:], 0.0)
            nc.vector.memset(h0T[:], 0.0)
            nc.vector.memset(h1T[:], 0.0)

            def wave_mms(pb, lhsTs, w_sb, ks, nk):
                """j-chunks run as 4 concurrent PE column groups sharing the
                stationary operand; K accumulates within each group, so psum
                group j ends up holding the complete [z_j|n_j] gates."""
                for k in ks:
                    for j in range(NJ):
                        nc.tensor.matmul(
                            pb[32 * j : 32 * j + 32, :],
                            lhsTs[k],
                            w_sb[:, k, j, :],
                            start=(k == 0),
                            stop=(k == nk - 1),
                            tile_position=(0, 32 * j),
                            skip_group_check=True,
                        )

            def layer_ew(pb, hg_prev, bias, htag):
                """Grouped elementwise in column halves + PE transposes;
                half 0 transposes while half 1 is still computing."""
                hg = state.tile([128, 256], BF16, tag=f"h{htag}g")
                hT = state.tile([128, NH, 32], BF16, tag=f"h{htag}T")
                nc.vector.memset(hT[:, :, BL:32], 0.0)
                for t2 in range(2):
                    cl, ch = 128 * t2, 128 * t2 + 128
                    n = tmp.tile([128, 128], F32, tag="n")
                    z = tmp.tile([128, 128], F32, tag="z")
                    if bias is not None:
                        zb = tmp.tile([128, 128], F32, tag="zb")
                        nb = tmp.tile([128, 128], F32, tag="nb")
                        nc.vector.tensor_add(nb[:], pb[:, 256 + cl : 256 + ch], bias[:, 256 + cl : 256 + ch])
                        nc.vector.tensor_add(zb[:], pb[:, cl:ch], bias[:, cl:ch])
                        nc.scalar.activation(n[:], nb[:], AF.Tanh)
                        nc.scalar.activation(z[:], zb[:], AF.Sigmoid)
                    else:
                        nc.scalar.activation(n[:], pb[:, 256 + cl : 256 + ch], AF.Tanh)
                        nc.scalar.activation(z[:], pb[:, cl:ch], AF.Sigmoid)
                    d = tmp.tile([128, 128], F32, tag="d")
                    m = tmp.tile([128, 128], F32, tag="m")
                    nc.vector.tensor_sub(d[:], hg_prev[:, cl:ch], n[:])
                    nc.vector.tensor_mul(m[:], z[:], d[:])
                    nc.vector.tensor_add(hg[:, cl:ch], n[:], m[:])
                    tp = tp_pool.tile([128, 128], BF16, tag="tp")
                    nc.tensor.transpose(tp[:], hg[:, cl:ch], id_sb[:])
                    for g in range(4):
                        c = 2 * g + t2
                        nc.vector.tensor_copy(
                            hT[:, c, 0:BL], tp[:, 32 * g : 32 * g + BL]
                        )
                return hg, hT

            # zero-pad columns of hT are relied on: memset above; transposes
            # only touch cols 0:16 of each chunk. xt pad likewise.
            b0s = bias_sb[0] if with_bias else None
            b1s = bias_sb[1] if with_bias else None

            for t in range(S):
                xst = xin.tile([128, BL], F32, tag="xst")
                nc.sync.dma_start(xst[:], xT_d[t])
                xtb = xin.tile([128, 32], BF16, tag="xtb")
                nc.vector.memset(xtb[:, BL:32], 0.0)
                nc.vector.tensor_copy(xtb[:, 0:BL], xst[:])

                l0 = [xtb[:]] + [h0T[:, c, :] for c in range(NH)]
                pb0 = pb_pool.tile([128, JW], F32, tag="pb")
                wave_mms(pb0, l0, w0_sb, range(K0), K0)

                # L1 h1-half first: depends only on last step's h1T, so it
                # runs on the PE while L0's elementwise chain drains.
                l1a = [h1T[:, c, :] for c in range(NH)]
                pb1 = pb_pool.tile([128, JW], F32, tag="pb")
                wave_mms(pb1, l1a, w1_sb, range(NH), K1)

                h0g, h0T = layer_ew(pb0, h0g, b0s, "0")

                l1 = l1a + [h0T[:, c, :] for c in range(NH)]
                wave_mms(pb1, l1, w1_sb, range(NH, K1), K1)
                h1g, h1T = layer_ew(pb1, h1g, b1s, "1")

            # ---- head: out = h1 @ Wfc + bfc ----
            php = pb_pool.tile([1, BL], F32, tag="ph")
            for c in range(NH):
                nc.tensor.matmul(
                    php[:],
                    wfc_sb[:, c : c + 1],
                    h1T[:, c, 0:BL],
                    start=(c == 0),
                    stop=(c == NH - 1),
                )
            o_sb = tmp.tile([1, BL], F32, tag="osb")
            nc.scalar.activation(o_sb[:], php[:], AF.Identity, bias=bfc_sb[:])
            nc.sync.dma_start(o_d[:], o_sb[:])
            xin_cm.__exit__(None, None, None)
            tmp_cm.__exit__(None, None, None)
            state_cm.__exit__(None, None, None)

    nc.compile()
    return nc


_CACHE = {}


def _get_nc(S, with_bias):
    key = (S, with_bias)
    if key not in _CACHE:
        _CACHE[key] = build_nc(S, with_bias)
    return _CACHE[key]


def _pack_inputs(x, W0, b0, W1, b1, Wfc, bfc):
    S = x.shape[1]
    H = HID
    # packed weights: [k, j, 128, 512] with cols [z_j(256) | n_j(256)]
    def pack(W, nk):
        out = np.empty((nk, NJ, 128, JW), dtype=np.float32)
        for k in range(nk):
            rows = W[k * 128 : (k + 1) * 128]
            for j in range(NJ):
                out[k, j, :, 0:256] = rows[:, j * 256 : (j + 1) * 256]
                out[k, j, :, 256:512] = rows[:, 2 * H + j * 256 : 2 * H + (j + 1) * 256]
        return out

    W0p = pack(W0, K0)
    # L1 stationary order is h1 chunks first (ready from the previous step),
    # then h0 chunks: reorder W1 rows to match.
    W1r = np.concatenate([W1[HID:], W1[:HID]], axis=0)
    W1p = pack(W1r, K1)
    idm = np.eye(128, dtype=np.float32)
    Wfcp = Wfc.reshape(NH, 128).astype(np.float32)

    def pack_bias(b):
        bg = np.zeros((128, JW), dtype=np.float32)
        for j in range(NJ):
            for bb in range(BL):
                bg[32 * j + bb, 0:256] = b[j * 256 : (j + 1) * 256]
                bg[32 * j + bb, 256:512] = b[2 * H + j * 256 : 2 * H + (j + 1) * 256]
        return bg

    b0g = pack_bias(b0)
    b1g = pack_bias(b1)

    in_maps = []
    for i in range(NCORES):
        xc = x[i * BL : (i + 1) * BL]  # (16, S, IN)
        xT = np.ascontiguousarray(xc.transpose(1, 2, 0))  # (S, IN, 16)
        in_maps.append(
            {
                "xT": xT,
                "W0p": W0p,
                "W1p": W1p,
                "Id": idm,
                "Wfcp": Wfcp,
                "bfc": bfc,
                "b0g": b0g,
                "b1g": b1g,
            }
        )
    return in_maps, S


def run(x, W0, b0, W1, b1, Wfc, bfc, **spmd_kwargs):
    from concourse.bass_utils import run_bass_kernel_spmd

    x = np.ascontiguousarray(np.asarray(x, dtype=np.float32))
    W0 = np.ascontiguousarray(np.asarray(W0, dtype=np.float32))
    W1 = np.ascontiguousarray(np.asarray(W1, dtype=np.float32))
    b0 = np.asarray(b0, dtype=np.float32)
    b1 = np.asarray(b1, dtype=np.float32)
    Wfc = np.ascontiguousarray(np.asarray(Wfc, dtype=np.float32))
    bfc = np.asarray(bfc, dtype=np.float32)

    in_maps, S = _pack_inputs(x, W0, b0, W1, b1, Wfc, bfc)
    with_bias = bool(np.any(b0) or np.any(b1))
    nc = _get_nc(S, with_bias)
    res = run_bass_kernel_spmd(
        nc, in_maps, core_ids=list(range(NCORES)), **spmd_kwargs
    )
    out = np.concatenate([np.asarray(r["o"]).reshape(BL) for r in res.results])
    return out.astype(np.float32), res


def kernel(x, W0, b0, W1, b1, Wfc, bfc):
    out, _ = run(x, W0, b0, W1, b1, Wfc, bfc)
    return out


# revision 8
# speedup vs baseline: 1.4798x; 1.4798x over previous
"""Trainium2 Bass kernel for nn_CustomSimpleGRU (2-layer GRU-like recurrence).

Reference math (per timestep t):
    L0: gates = [x_t, h0] @ W0 + b0 ; z = sigmoid(gates[:, :H]) ; n = tanh(gates[:, 2H:3H])
        h0' = (1-z)*n + z*h0
    L1: gates = [h0', h1] @ W1 + b1 ; same gate math -> h1'
    out = h1'(last step) @ Wfc + bfc     (reset-gate chunk [H:2H] never used)

Sharding: data-parallel over batch (128 -> 16 per core x 8 cores), weights
replicated, time recurrence fully unrolled per core.

Per-core scheme ("column-packed" matmuls to beat the M=16 PE ceiling):
  - All matmuls use M=32 (batch 16 + 16 zero-padded lanes).
  - Gate columns are packed host-side as j-chunks: rhs_j = [z_j(256) | n_j(256)],
    j = 0..3, so one 512-wide matmul produces matching z/n slices.
  - For each j, the K-chunks (9 for L0, 16 for L1) are distributed over the
    four 32-row PE column groups via tile_position=(0, 32g): 4 concurrent
    accumulations -> psumA row-group g holds the partial sum of its K-subset.
  - The 4 partials are folded by one matmul with a 0/1 selection matrix
    Sel (Sel[p, b] = [p % 32 == b]) into row-group j of psumB: psumB then
    holds the complete layer gates for all four j-chunks, grouped
    [32*j + b] rows x [z(256) | n(256)] cols.
  - Elementwise (sigmoid/tanh/h-update) runs on full (128, 256) tiles
    (junk lanes compute zeros, discarded).
  - h state is kept in the same grouped layout h_g (128, 256):
    row 32*g + b, col f  =  h[b, 256*g + f].
  - Next step's stationary operands hT (128, 16) per 128-col h-chunk are made
    by 8 DMA-transposes per layer from h_g slices (off the PE).
  - Weight K-chunk c pairs hT chunk c = h cols [128c : 128c+128]; host packs
    W accordingly (natural order).
"""

import numpy as np

import concourse.bass as bass
import concourse.mybir as mybir
import concourse.tile as tile
from concourse import bacc

F32 = mybir.dt.float32
BF16 = mybir.dt.bfloat16
AF = mybir.ActivationFunctionType

B, S_FULL, IN, HID = 128, 512, 128, 1024
NCORES = 8
BL = B // NCORES  # 16 batch rows per core
NH = HID // 128  # 8 h-dim chunks
NJ = 4  # gate j-chunks, each [z 256 | n 256]
JW = 512  # packed j-chunk width
K0, K1 = 1 + NH, 2 * NH  # K-chunks per layer


def build_nc(S=S_FULL, with_bias=False):
    nc = bacc.Bacc("TRN2")
    xT_d = nc.dram_tensor("xT", [S, IN, BL], F32, kind="ExternalInput")
    w0_d = nc.dram_tensor("W0p", [K0, NJ, 128, JW], F32, kind="ExternalInput")
    w1_d = nc.dram_tensor("W1p", [K1, NJ, 128, JW], F32, kind="ExternalInput")
    id_d = nc.dram_tensor("Id", [128, 128], F32, kind="ExternalInput")
    wfc_d = nc.dram_tensor("Wfcp", [NH, 128], F32, kind="ExternalInput")
    bfc_d = nc.dram_tensor("bfc", [1], F32, kind="ExternalInput")
    b0_d = nc.dram_tensor("b0g", [128, JW], F32, kind="ExternalInput")
    b1_d = nc.dram_tensor("b1g", [128, JW], F32, kind="ExternalInput")
    o_d = nc.dram_tensor("o", [1, BL], F32, kind="ExternalOutput")

    with tile.TileContext(nc) as tc:
        with (
            tc.tile_pool(name="wts", bufs=1) as wts,
            tc.tile_pool(name="pb", bufs=3, space="PSUM") as pb_pool,
            tc.tile_pool(name="tp", bufs=2, space="PSUM") as tp_pool,
        ):
            stage_cm = tc.tile_pool(name="stage", bufs=2)
            stage = stage_cm.__enter__()
            # ---- weights fp32 DRAM -> bf16 SBUF ----
            w0_sb = wts.tile([128, K0, NJ, JW], BF16, tag="w0")
            w1_sb = wts.tile([128, K1, NJ, JW], BF16, tag="w1")
            for w_sb, w_d, kk in ((w0_sb, w0_d, K0), (w1_sb, w1_d, K1)):
                for k in range(kk):
                    for j in range(NJ):
                        st = stage.tile([128, JW], F32, tag="wstage")
                        nc.sync.dma_start(st[:], w_d[k, j])
                        nc.vector.tensor_copy(w_sb[:, k, j, :], st[:])
            id_sb = wts.tile([128, 128], BF16, tag="idm")
            id_st = stage.tile([128, 128], F32, tag="idst")
            nc.sync.dma_start(id_st[:], id_d[:])
            nc.vector.tensor_copy(id_sb[:], id_st[:])
            wfc_sb = wts.tile([128, NH], BF16, tag="wfc")
            wfc_st = stage.tile([128, NH], F32, tag="wfcst")
            wfc_ap = wfc_d[:]
            nc.sync.dma_start(
                wfc_st[:],
                bass.AP(tensor=wfc_ap.tensor, offset=0, ap=[[1, 128], [128, NH]]),
            )
            nc.vector.tensor_copy(wfc_sb[:], wfc_st[:])
            bfc_sb = wts.tile([1, 1], F32, tag="bfc")
            nc.sync.dma_start(bfc_sb[:], bfc_d[:])
            bias_sb = []
            if with_bias:
                for nm, b_d in (("b0", b0_d), ("b1", b1_d)):
                    bt = wts.tile([128, JW], F32, tag=f"bias{nm}", name=f"bias{nm}")
                    nc.sync.dma_start(bt[:], b_d[:])
                    bias_sb.append(bt)

            stage_cm.__exit__(None, None, None)
            state_cm = tc.tile_pool(name="state", bufs=2)
            state = state_cm.__enter__()
            tmp_cm = tc.tile_pool(name="tmp", bufs=2)
            tmp = tmp_cm.__enter__()
            xin_cm = tc.tile_pool(name="xin", bufs=4)
            xin = xin_cm.__enter__()

            # ---- initial state ----
            h0g = state.tile([128, 256], BF16, tag="h0g")
            h1g = state.tile([128, 256], BF16, tag="h1g")
            h0T = state.tile([128, 2, 4, 32], BF16, tag="h0T")
            h1T = state.tile([128, 2, 4, 32], BF16, tag="h1T")
            nc.vector.memset(h0g[:], 0.0)
            nc.vector.memset(h1g[:], 0.0)
            nc.vector.memset(h0T[:], 0.0)
            nc.vector.memset(h1T[:], 0.0)

            def wave_mms(pb, lhsTs, w_sb, ks, nk):
                """j-chunks run as 4 concurrent PE column groups sharing the
                stationary operand; K accumulates within each group, so psum
                group j ends up holding the complete [z_j|n_j] gates."""
                for k in ks:
                    for j in range(NJ):
                        nc.tensor.matmul(
                            pb[32 * j : 32 * j + 32, :],
                            lhsTs[k],
                            w_sb[:, k, j, :],
                            start=(k == 0),
                            stop=(k == nk - 1),
                            tile_position=(0, 32 * j),
                            skip_group_check=True,
                        )

            def layer_ew(pb, hg_prev, bias, htag):
                """Grouped elementwise in column halves + PE transposes;
                half 0 transposes while half 1 is still computing."""
                hg = state.tile([128, 256], BF16, tag=f"h{htag}g")
                hT = state.tile([128, 2, 4, 32], BF16, tag=f"h{htag}T")
                nc.vector.memset(hT[:, :, :, BL:32], 0.0)
                for t2 in range(2):
                    cl, ch = 128 * t2, 128 * t2 + 128
                    n = tmp.tile([128, 128], F32, tag="n")
                    z = tmp.tile([128, 128], F32, tag="z")
                    if bias is not None:
                        zb = tmp.tile([128, 128], F32, tag="zb")
                        nb = tmp.tile([128, 128], F32, tag="nb")
                        nc.vector.tensor_add(nb[:], pb[:, 256 + cl : 256 + ch], bias[:, 256 + cl : 256 + ch])
                        nc.vector.tensor_add(zb[:], pb[:, cl:ch], bias[:, cl:ch])
                        nc.scalar.activation(n[:], nb[:], AF.Tanh)
                        nc.scalar.activation(z[:], zb[:], AF.Sigmoid)
                    else:
                        nc.scalar.activation(n[:], pb[:, 256 + cl : 256 + ch], AF.Tanh)
                        nc.scalar.activation(z[:], pb[:, cl:ch], AF.Sigmoid)
                    d = tmp.tile([128, 128], F32, tag="d")
                    m = tmp.tile([128, 128], F32, tag="m")
                    nc.vector.tensor_sub(d[:], hg_prev[:, cl:ch], n[:])
                    nc.vector.tensor_mul(m[:], z[:], d[:])
                    nc.vector.tensor_add(hg[:, cl:ch], n[:], m[:])
                    tp = tp_pool.tile([128, 128], BF16, tag="tp")
                    nc.tensor.transpose(tp[:], hg[:, cl:ch], id_sb[:])
                    src = bass.AP(
                        tensor=tp[:].tensor,
                        offset=tp[:].offset,
                        ap=[tp[:].ap[0], [32, 4], [1, BL]],
                    )
                    nc.vector.tensor_copy(hT[:, t2, :, 0:BL], src)
                return hg, hT

            # zero-pad columns of hT are relied on: memset above; transposes
            # only touch cols 0:16 of each chunk. xt pad likewise.
            b0s = bias_sb[0] if with_bias else None
            b1s = bias_sb[1] if with_bias else None
            pb1_prev = None

            for t in range(S):
                xst = xin.tile([128, BL], F32, tag="xst")
                nc.sync.dma_start(xst[:], xT_d[t])
                xtb = xin.tile([128, 32], BF16, tag="xtb")
                nc.vector.memset(xtb[:, BL:32], 0.0)
                nc.vector.tensor_copy(xtb[:, 0:BL], xst[:])

                l0 = [xtb[:]] + [h0T[:, c % 2, c // 2, :] for c in range(NH)]
                pb0 = pb_pool.tile([128, JW], F32, tag="pb")
                wave_mms(pb0, l0, w0_sb, range(K0), K0)

                # Deferred L1(t-1) elementwise: its chain hides under the
                # L0(t) waves just emitted; its transposes precede the
                # L1(t) h1-half waves that consume h1T.
                if pb1_prev is not None:
                    h1g, h1T = layer_ew(pb1_prev, h1g, b1s, "1")

                # L1 h1-half: depends only on the just-produced h1T.
                l1a = [h1T[:, c % 2, c // 2, :] for c in range(NH)]
                pb1 = pb_pool.tile([128, JW], F32, tag="pb")
                wave_mms(pb1, l1a, w1_sb, range(NH), K1)

                h0g, h0T = layer_ew(pb0, h0g, b0s, "0")

                l1 = l1a + [h0T[:, c % 2, c // 2, :] for c in range(NH)]
                wave_mms(pb1, l1, w1_sb, range(NH, K1), K1)
                pb1_prev = pb1

            # final deferred L1 elementwise
            h1g, h1T = layer_ew(pb1_prev, h1g, b1s, "1")
            # ---- head: out = h1 @ Wfc + bfc ----
            php = tp_pool.tile([1, BL], F32, tag="ph")
            for c in range(NH):
                nc.tensor.matmul(
                    php[:],
                    wfc_sb[:, c : c + 1],
                    h1T[:, c % 2, c // 2, 0:BL],
                    start=(c == 0),
                    stop=(c == NH - 1),
                )
            o_sb = tmp.tile([1, BL], F32, tag="osb")
            nc.scalar.activation(o_sb[:], php[:], AF.Identity, bias=bfc_sb[:])
            nc.sync.dma_start(o_d[:], o_sb[:])
            xin_cm.__exit__(None, None, None)
            tmp_cm.__exit__(None, None, None)
            state_cm.__exit__(None, None, None)

    nc.compile()
    return nc


_CACHE = {}


def _get_nc(S, with_bias):
    key = (S, with_bias)
    if key not in _CACHE:
        _CACHE[key] = build_nc(S, with_bias)
    return _CACHE[key]


def _pack_inputs(x, W0, b0, W1, b1, Wfc, bfc):
    S = x.shape[1]
    H = HID
    # packed weights: [k, j, 128, 512] with cols [z_j(256) | n_j(256)]
    def pack(W, nk):
        out = np.empty((nk, NJ, 128, JW), dtype=np.float32)
        for k in range(nk):
            rows = W[k * 128 : (k + 1) * 128]
            for j in range(NJ):
                out[k, j, :, 0:256] = rows[:, j * 256 : (j + 1) * 256]
                out[k, j, :, 256:512] = rows[:, 2 * H + j * 256 : 2 * H + (j + 1) * 256]
        return out

    W0p = pack(W0, K0)
    # L1 stationary order is h1 chunks first (ready from the previous step),
    # then h0 chunks: reorder W1 rows to match.
    W1r = np.concatenate([W1[HID:], W1[:HID]], axis=0)
    W1p = pack(W1r, K1)
    idm = np.eye(128, dtype=np.float32)
    Wfcp = Wfc.reshape(NH, 128).astype(np.float32)

    def pack_bias(b):
        bg = np.zeros((128, JW), dtype=np.float32)
        for j in range(NJ):
            for bb in range(BL):
                bg[32 * j + bb, 0:256] = b[j * 256 : (j + 1) * 256]
                bg[32 * j + bb, 256:512] = b[2 * H + j * 256 : 2 * H + (j + 1) * 256]
        return bg

    b0g = pack_bias(b0)
    b1g = pack_bias(b1)

    in_maps = []
    for i in range(NCORES):
        xc = x[i * BL : (i + 1) * BL]  # (16, S, IN)
        xT = np.ascontiguousarray(xc.transpose(1, 2, 0))  # (S, IN, 16)
        in_maps.append(
            {
                "xT": xT,
                "W0p": W0p,
                "W1p": W1p,
                "Id": idm,
                "Wfcp": Wfcp,
                "bfc": bfc,
                "b0g": b0g,
                "b1g": b1g,
            }
        )
    return in_maps, S


def run(x, W0, b0, W1, b1, Wfc, bfc, **spmd_kwargs):
    from concourse.bass_utils import run_bass_kernel_spmd

    x = np.ascontiguousarray(np.asarray(x, dtype=np.float32))
    W0 = np.ascontiguousarray(np.asarray(W0, dtype=np.float32))
    W1 = np.ascontiguousarray(np.asarray(W1, dtype=np.float32))
    b0 = np.asarray(b0, dtype=np.float32)
    b1 = np.asarray(b1, dtype=np.float32)
    Wfc = np.ascontiguousarray(np.asarray(Wfc, dtype=np.float32))
    bfc = np.asarray(bfc, dtype=np.float32)

    in_maps, S = _pack_inputs(x, W0, b0, W1, b1, Wfc, bfc)
    with_bias = bool(np.any(b0) or np.any(b1))
    nc = _get_nc(S, with_bias)
    res = run_bass_kernel_spmd(
        nc, in_maps, core_ids=list(range(NCORES)), **spmd_kwargs
    )
    out = np.concatenate([np.asarray(r["o"]).reshape(BL) for r in res.results])
    return out.astype(np.float32), res


def kernel(x, W0, b0, W1, b1, Wfc, bfc):
    out, _ = run(x, W0, b0, W1, b1, Wfc, bfc)
    return out
